# revision 2
# baseline (speedup 1.0000x reference)
"""Bahdanau additive attention on 8 Trainium2 NeuronCores.

Computation (per batch b, all fp32):
    Ws = enc @ W_a                    [Te, De]
    Uh = dec @ U_a                    [Td, De]
    scores[q, t] = V . tanh(Ws[t] + Uh[q])
    e = softmax(scores, axis=t)
    c = e @ enc

Sharding: pure data-parallel over batch B=8 -> one batch element per core.

Per-core engine split:
  - DVE:  broadcast adds  A[f, (q,t)] = WsT[f,t] + UhT[f,q]  (tensor_scalar, 2x fp32 mode)
  - ACT:  tanh over big [128, NQ*256] tiles (amortizes the 222-cycle ACTIVATE overhead)
  - PE:   V-contraction via a shifted-window weight matrix ("Vz" trick): for
          in-block decoder step jj, lhsT = Vz[:, 127-jj : 255-jj] has V in
          column jj and zeros elsewhere, so the matmul deposits the score row
          for step jj into partition jj of the PSUM scores tile (all other
          rows accumulate += 0).  Also computes WsT/UhT setup and e @ enc.
"""

import sys

import numpy as np

for _p in ("/opt/trn_rl_repo",):
    if _p not in sys.path:
        sys.path.insert(0, _p)

B = 8
TE = 256
TD = 256
DE = 512
KCH = DE // 128          # feature chunks of 128 partitions
QB = TD // 128           # q-blocks of 128 decoder steps
NQ = 16                  # decoder steps per tanh super-instruction
NGRP = 128 // NQ

_CACHE = {}


def _build_program():
    from contextlib import ExitStack

    import concourse.bacc as bacc
    import concourse.tile as tile
    from concourse import mybir

    f32 = mybir.dt.float32
    AF = mybir.ActivationFunctionType
    ALU = mybir.AluOpType
    AX = mybir.AxisListType

    nc = bacc.Bacc("TRN2", target_bir_lowering=False, debug=False, num_devices=B)

    encT = nc.dram_tensor("encT", [DE, TE], f32, kind="ExternalInput").ap()
    decT = nc.dram_tensor("decT", [DE, TD], f32, kind="ExternalInput").ap()
    enc = nc.dram_tensor("enc", [TE, DE], f32, kind="ExternalInput").ap()
    W = nc.dram_tensor("W", [DE, DE], f32, kind="ExternalInput").ap()
    U = nc.dram_tensor("U", [DE, DE], f32, kind="ExternalInput").ap()
    Vz = nc.dram_tensor("Vz", [128, KCH * 255], f32, kind="ExternalInput").ap()
    ident = nc.dram_tensor("ident", [128, 128], f32, kind="ExternalInput").ap()
    c_out = nc.dram_tensor("c_out", [TD, DE], f32, kind="ExternalOutput").ap()
    e_out = nc.dram_tensor("e_out", [TD, TE], f32, kind="ExternalOutput").ap()

    with tile.TileContext(nc) as tc:
        with ExitStack() as ctx:
            consts = ctx.enter_context(tc.tile_pool(name="consts", bufs=1))
            apool = ctx.enter_context(tc.tile_pool(name="apool", bufs=2))
            tpool = ctx.enter_context(tc.tile_pool(name="tpool", bufs=2))
            spool = ctx.enter_context(tc.tile_pool(name="spool", bufs=2))
            epool = ctx.enter_context(tc.tile_pool(name="epool", bufs=2))
            ps_setup = ctx.enter_context(
                tc.tile_pool(name="ps_setup", bufs=2, space="PSUM")
            )
            ps_scores = ctx.enter_context(
                tc.tile_pool(name="ps_scores", bufs=2, space="PSUM")
            )
            ps_tr = ctx.enter_context(tc.tile_pool(name="ps_tr", bufs=2, space="PSUM"))
            ps_ctx = ctx.enter_context(tc.tile_pool(name="ps_ctx", bufs=2, space="PSUM"))

            # ---- load constants into SBUF ----
            et, dt_, wt, ut = [], [], [], []
            for k in range(KCH):
                t = consts.tile([128, TE], f32, tag=f"et{k}")
                nc.sync.dma_start(out=t[:], in_=encT[k * 128:(k + 1) * 128, :])
                et.append(t)
                t = consts.tile([128, TD], f32, tag=f"dt{k}")
                nc.sync.dma_start(out=t[:], in_=decT[k * 128:(k + 1) * 128, :])
                dt_.append(t)
                t = consts.tile([128, DE], f32, tag=f"wt{k}")
                nc.sync.dma_start(out=t[:], in_=W[k * 128:(k + 1) * 128, :])
                wt.append(t)
                t = consts.tile([128, DE], f32, tag=f"ut{k}")
                nc.sync.dma_start(out=t[:], in_=U[k * 128:(k + 1) * 128, :])
                ut.append(t)
            enc_t = []
            for th in range(TE // 128):
                t = consts.tile([128, DE], f32, tag=f"enc{th}")
                nc.sync.dma_start(out=t[:], in_=enc[th * 128:(th + 1) * 128, :])
                enc_t.append(t)
            vz = consts.tile([128, KCH * 255], f32, tag="vz")
            nc.sync.dma_start(out=vz[:], in_=Vz[:, :])
            idt = consts.tile([128, 128], f32, tag="idt")
            nc.sync.dma_start(out=idt[:], in_=ident[:, :])

            # ---- WsT[f, t] and UhT[f, q] (feature-major layouts) ----
            wst, uht = [], []
            for m in range(KCH):
                ps = ps_setup.tile([128, TE], f32, tag="ps_setup")
                for k in range(KCH):
                    nc.tensor.matmul(
                        ps[:],
                        wt[k][:, m * 128:(m + 1) * 128],
                        et[k][:],
                        start=(k == 0),
                        stop=(k == KCH - 1),
                    )
                t = consts.tile([128, TE], f32, tag=f"wst{m}")
                nc.scalar.copy(t[:], ps[:])
                wst.append(t)
            for m in range(KCH):
                ps = ps_setup.tile([128, TD], f32, tag="ps_setup")
                for k in range(KCH):
                    nc.tensor.matmul(
                        ps[:],
                        ut[k][:, m * 128:(m + 1) * 128],
                        dt_[k][:],
                        start=(k == 0),
                        stop=(k == KCH - 1),
                    )
                t = consts.tile([128, TD], f32, tag=f"uht{m}")
                nc.scalar.copy(t[:], ps[:])
                uht.append(t)

            # ---- main loop over q-blocks of 128 decoder steps ----
            for qb in range(QB):
                scores = ps_scores.tile([128, TE], f32, tag="scores")
                n_mm = 0
                for g in range(NGRP):
                    for k in range(KCH):
                        a = apool.tile([128, NQ, TE], f32, tag="a")
                        for j in range(NQ):
                            q = qb * 128 + g * NQ + j
                            nc.vector.tensor_scalar_add(
                                a[:, j, :], wst[k][:], uht[k][:, q:q + 1]
                            )
                        tt = tpool.tile([128, NQ, TE], f32, tag="t")
                        nc.scalar.activation(tt[:], a[:], AF.Tanh)
                        for j in range(NQ):
                            jj = g * NQ + j
                            nc.tensor.matmul(
                                scores[:],
                                vz[:, k * 255 + 127 - jj: k * 255 + 255 - jj],
                                tt[:, j, :],
                                start=(n_mm == 0),
                                stop=(n_mm == 128 * KCH - 1),
                            )
                            n_mm += 1

                # softmax along t (free dim)
                negmax = spool.tile([128, 1], f32, tag="negmax")
                nc.vector.tensor_reduce(
                    negmax[:], scores[:], axis=AX.X, op=ALU.max, negate=True
                )
                e_raw = epool.tile([128, TE], f32, tag="e_raw")
                ssum = spool.tile([128, 1], f32, tag="ssum")
                nc.scalar.activation(
                    e_raw[:], scores[:], AF.Exp, bias=negmax[:], accum_out=ssum[:]
                )
                rsum = spool.tile([128, 1], f32, tag="rsum")
                nc.vector.reciprocal(rsum[:], ssum[:])
                e_norm = epool.tile([128, TE], f32, tag="e_norm")
                nc.vector.tensor_scalar_mul(e_norm[:], e_raw[:], rsum[:])
                nc.sync.dma_start(
                    out=e_out[qb * 128:(qb + 1) * 128, :], in_=e_norm[:]
                )

                # context c[q, :] = sum_t e[q, t] * enc[t, :]
                etr = []
                for th in range(TE // 128):
                    trp = ps_tr.tile([128, 128], f32, tag="trp")
                    nc.tensor.transpose(
                        trp[:], e_norm[:, th * 128:(th + 1) * 128], idt[:]
                    )
                    t = epool.tile([128, 128], f32, tag=f"etr{th}")
                    nc.scalar.copy(t[:], trp[:])
                    etr.append(t)
                cps = ps_ctx.tile([128, DE], f32, tag="cps")
                for th in range(TE // 128):
                    nc.tensor.matmul(
                        cps[:],
                        etr[th][:],
                        enc_t[th][:],
                        start=(th == 0),
                        stop=(th == TE // 128 - 1),
                    )
                c_sb = epool.tile([128, DE], f32, tag="c_sb")
                nc.scalar.copy(c_sb[:], cps[:])
                nc.sync.dma_start(
                    out=c_out[qb * 128:(qb + 1) * 128, :], in_=c_sb[:]
                )

    nc.compile()
    return nc


def _get_program():
    if "nc" not in _CACHE:
        _CACHE["nc"] = _build_program()
    return _CACHE["nc"]


def _prep_in_maps(encoder_out_seq, decoder_out_seq, W_a, U_a, V_a):
    enc_full = np.ascontiguousarray(np.asarray(encoder_out_seq, dtype=np.float32))
    dec_full = np.ascontiguousarray(np.asarray(decoder_out_seq, dtype=np.float32))
    W = np.ascontiguousarray(np.asarray(W_a, dtype=np.float32))
    U = np.ascontiguousarray(np.asarray(U_a, dtype=np.float32))
    V = np.asarray(V_a, dtype=np.float32).reshape(DE)

    vz = np.zeros((128, KCH * 255), dtype=np.float32)
    for k in range(KCH):
        vz[:, k * 255 + 127] = V[k * 128:(k + 1) * 128]
    ident = np.eye(128, dtype=np.float32)

    in_maps = []
    for b in range(B):
        in_maps.append(
            {
                "encT": np.ascontiguousarray(enc_full[b].T),
                "decT": np.ascontiguousarray(dec_full[b].T),
                "enc": enc_full[b],
                "W": W,
                "U": U,
                "Vz": vz,
                "ident": ident,
            }
        )
    return in_maps


def run(inputs, trace=False):
    from concourse.bass_utils import run_bass_kernel_spmd

    nc = _get_program()
    in_maps = _prep_in_maps(**inputs)
    res = run_bass_kernel_spmd(nc, in_maps, list(range(B)), trace=trace)
    c = np.stack([res.results[b]["c_out"] for b in range(B)])
    e = np.stack([res.results[b]["e_out"] for b in range(B)])
    return (c, e), res


def predict_ns(trace_path=None):
    """Cost-model timeline estimate for one core (no HW profiling needed)."""
    from concourse.timeline_sim import TimelineSim

    nc = _get_program()
    sim = TimelineSim(nc, trace=trace_path is not None)
    t = sim.simulate()
    if trace_path is not None and sim.perfetto is not None:
        sim.perfetto.save(trace_path)
    return t


def kernel(**inputs):
    out, _ = run(inputs, trace=False)
    return out


# revision 3
# speedup vs baseline: 1.0017x; 1.0017x over previous
"""Bahdanau additive attention on 8 Trainium2 NeuronCores.

Computation (per batch b, all fp32):
    Ws = enc @ W_a                    [Te, De]
    Uh = dec @ U_a                    [Td, De]
    scores[q, t] = V . tanh(Ws[t] + Uh[q])
    e = softmax(scores, axis=t)
    c = e @ enc

Sharding: pure data-parallel over batch B=8 -> one batch element per core.

Per-core engine split:
  - DVE:  broadcast adds  A[f, (q,t)] = WsT[f,t] + UhT[f,q]  (tensor_scalar,
          2x fp32 SBUF mode), plus all PSUM->SBUF copies.
  - ACT:  tanh over big [128, NQ*256] tiles (amortizes the 222-cycle
          ACTIVATE overhead), softmax exp with fused bias and sum-accum.
  - PE:   V-contraction via shifted-window 32-column weights: the scores
          row for in-block step jj lives at PSUM partition
          sigma(jj) = (jj%4)*32 + jj//4, so consecutive matmuls hit the four
          PE column-groups round-robin and execute concurrently
          (tile_position col tiling).  Also WsT/UhT setup and e @ enc.

The row permutation is undone by the output DMA access pattern.
"""

import sys

import numpy as np

for _p in ("/opt/trn_rl_repo",):
    if _p not in sys.path:
        sys.path.insert(0, _p)

B = 8
TE = 256
TD = 256
DE = 512
KCH = DE // 128          # feature chunks of 128 partitions
QB = TD // 128           # q-blocks of 128 decoder steps
NQ = 32                  # decoder steps per tanh super-instruction
NGRP = 128 // NQ

_CACHE = {}


def _build_program():
    from contextlib import ExitStack

    import concourse.bacc as bacc
    import concourse.tile as tile
    from concourse import mybir

    f32 = mybir.dt.float32
    AF = mybir.ActivationFunctionType
    ALU = mybir.AluOpType
    AX = mybir.AxisListType

    nc = bacc.Bacc("TRN2", target_bir_lowering=False, debug=False, num_devices=B)

    encT = nc.dram_tensor("encT", [DE, TE], f32, kind="ExternalInput").ap()
    decT = nc.dram_tensor("decT", [DE, TD], f32, kind="ExternalInput").ap()
    enc = nc.dram_tensor("enc", [TE, DE], f32, kind="ExternalInput").ap()
    W = nc.dram_tensor("W", [DE, DE], f32, kind="ExternalInput").ap()
    U = nc.dram_tensor("U", [DE, DE], f32, kind="ExternalInput").ap()
    Vz = nc.dram_tensor("Vz", [128, KCH * 63], f32, kind="ExternalInput").ap()
    ident = nc.dram_tensor("ident", [128, 128], f32, kind="ExternalInput").ap()
    c_out = nc.dram_tensor("c_out", [TD, DE], f32, kind="ExternalOutput").ap()
    e_out = nc.dram_tensor("e_out", [TD, TE], f32, kind="ExternalOutput").ap()

    with tile.TileContext(nc) as tc:
        with ExitStack() as ctx:
            consts = ctx.enter_context(tc.tile_pool(name="consts", bufs=1))
            apool = ctx.enter_context(tc.tile_pool(name="apool", bufs=2))
            tpool = ctx.enter_context(tc.tile_pool(name="tpool", bufs=2))
            spool = ctx.enter_context(tc.tile_pool(name="spool", bufs=2))
            epool = ctx.enter_context(tc.tile_pool(name="epool", bufs=2))
            ps_setup = ctx.enter_context(
                tc.tile_pool(name="ps_setup", bufs=2, space="PSUM")
            )
            ps_scores = ctx.enter_context(
                tc.tile_pool(name="ps_scores", bufs=2, space="PSUM")
            )
            ps_tr = ctx.enter_context(tc.tile_pool(name="ps_tr", bufs=2, space="PSUM"))
            ps_ctx = ctx.enter_context(tc.tile_pool(name="ps_ctx", bufs=2, space="PSUM"))

            # ---- load constants into SBUF ----
            et, dt_, wt, ut = [], [], [], []
            for k in range(KCH):
                t = consts.tile([128, TE], f32, tag=f"et{k}")
                nc.sync.dma_start(out=t[:], in_=encT[k * 128:(k + 1) * 128, :])
                et.append(t)
                t = consts.tile([128, TD], f32, tag=f"dt{k}")
                nc.sync.dma_start(out=t[:], in_=decT[k * 128:(k + 1) * 128, :])
                dt_.append(t)
                t = consts.tile([128, DE], f32, tag=f"wt{k}")
                nc.sync.dma_start(out=t[:], in_=W[k * 128:(k + 1) * 128, :])
                wt.append(t)
                t = consts.tile([128, DE], f32, tag=f"ut{k}")
                nc.sync.dma_start(out=t[:], in_=U[k * 128:(k + 1) * 128, :])
                ut.append(t)
            enc_t = []
            for th in range(TE // 128):
                t = consts.tile([128, DE], f32, tag=f"enc{th}")
                nc.sync.dma_start(out=t[:], in_=enc[th * 128:(th + 1) * 128, :])
                enc_t.append(t)
            vz = consts.tile([128, KCH * 63], f32, tag="vz")
            nc.sync.dma_start(out=vz[:], in_=Vz[:, :])
            idt = consts.tile([128, 128], f32, tag="idt")
            nc.sync.dma_start(out=idt[:], in_=ident[:, :])

            # ---- WsT[f, t] and UhT[f, q] (feature-major layouts) ----
            wst, uht = [], []
            for m in range(KCH):
                ps = ps_setup.tile([128, TE], f32, tag="ps_setup")
                for k in range(KCH):
                    nc.tensor.matmul(
                        ps[:],
                        wt[k][:, m * 128:(m + 1) * 128],
                        et[k][:],
                        start=(k == 0),
                        stop=(k == KCH - 1),
                    )
                t = consts.tile([128, TE], f32, tag=f"wst{m}")
                nc.vector.tensor_copy(t[:], ps[:])
                wst.append(t)
            for m in range(KCH):
                ps = ps_setup.tile([128, TD], f32, tag="ps_setup")
                for k in range(KCH):
                    nc.tensor.matmul(
                        ps[:],
                        ut[k][:, m * 128:(m + 1) * 128],
                        dt_[k][:],
                        start=(k == 0),
                        stop=(k == KCH - 1),
                    )
                t = consts.tile([128, TD], f32, tag=f"uht{m}")
                nc.vector.tensor_copy(t[:], ps[:])
                uht.append(t)

            # ---- main loop over q-blocks of 128 decoder steps ----
            # In-block step jj lives at PSUM/SBUF row sigma(jj):
            #   sigma(jj) = (jj % 4) * 32 + jj // 4
            # so consecutive jj hit distinct PE column groups.
            for qb in range(QB):
                scores = ps_scores.tile([128, TE], f32, tag="scores")
                for g in range(NGRP):
                    for k in range(KCH):
                        a = apool.tile([128, NQ, TE], f32, tag="a")
                        for j in range(NQ):
                            q = qb * 128 + g * NQ + j
                            nc.vector.tensor_scalar_add(
                                a[:, j, :], wst[k][:], uht[k][:, q:q + 1]
                            )
                        tt = tpool.tile([128, NQ, TE], f32, tag="t")
                        nc.scalar.activation(tt[:], a[:], AF.Tanh)
                        for j in range(NQ):
                            jj = g * NQ + j
                            cg = jj % 4       # column group
                            i = jj // 4       # row inside the column group
                            first = g == 0 and k == 0 and j < 4
                            last = g == NGRP - 1 and k == KCH - 1 and j >= NQ - 4
                            nc.tensor.matmul(
                                scores[32 * cg:32 * (cg + 1), :],
                                vz[:, k * 63 + 31 - i: k * 63 + 63 - i],
                                tt[:, j, :],
                                start=first,
                                stop=last,
                                tile_position=(0, 32 * cg),
                            )

                # softmax along t (free dim); rows are sigma-permuted
                negmax = spool.tile([128, 1], f32, tag="negmax")
                nc.vector.tensor_reduce(
                    negmax[:], scores[:], axis=AX.X, op=ALU.max, negate=True
                )
                e_raw = epool.tile([128, TE], f32, tag="e_raw")
                ssum = spool.tile([128, 1], f32, tag="ssum")
                nc.scalar.activation(
                    e_raw[:], scores[:], AF.Exp, bias=negmax[:], accum_out=ssum[:]
                )
                rsum = spool.tile([128, 1], f32, tag="rsum")
                nc.vector.reciprocal(rsum[:], ssum[:])
                e_norm = epool.tile([128, TE], f32, tag="e_norm")
                nc.vector.tensor_scalar_mul(e_norm[:], e_raw[:], rsum[:])
                # undo the row permutation in the DMA: SBUF row r holds DRAM
                # row 4*(r%32) + r//32, i.e. DRAM rows iterate as
                # (outer r//32: stride 1 row) x (inner r%32: stride 4 rows).
                eblk = e_out[qb * 128:(qb + 1) * 128, :]
                nc.sync.dma_start(
                    out=eblk.rearrange("(a b) t -> b a t", b=4), in_=e_norm[:]
                )

                # context c[q, :] = sum_t e[q, t] * enc[t, :]
                etr = []
                for th in range(TE // 128):
                    trp = ps_tr.tile([128, 128], f32, tag="trp")
                    nc.tensor.transpose(
                        trp[:], e_norm[:, th * 128:(th + 1) * 128], idt[:]
                    )
                    t = epool.tile([128, 128], f32, tag=f"etr{th}")
                    nc.vector.tensor_copy(t[:], trp[:])
                    etr.append(t)
                cps = ps_ctx.tile([128, DE], f32, tag="cps")
                for th in range(TE // 128):
                    nc.tensor.matmul(
                        cps[:],
                        etr[th][:],
                        enc_t[th][:],
                        start=(th == 0),
                        stop=(th == TE // 128 - 1),
                    )
                c_sb = epool.tile([128, DE], f32, tag="c_sb")
                nc.vector.tensor_copy(c_sb[:], cps[:])
                cblk = c_out[qb * 128:(qb + 1) * 128, :]
                nc.sync.dma_start(
                    out=cblk.rearrange("(a b) t -> b a t", b=4), in_=c_sb[:]
                )

    nc.compile()
    return nc


def _get_program():
    if "nc" not in _CACHE:
        _CACHE["nc"] = _build_program()
    return _CACHE["nc"]


def _prep_in_maps(encoder_out_seq, decoder_out_seq, W_a, U_a, V_a):
    enc_full = np.ascontiguousarray(np.asarray(encoder_out_seq, dtype=np.float32))
    dec_full = np.ascontiguousarray(np.asarray(decoder_out_seq, dtype=np.float32))
    W = np.ascontiguousarray(np.asarray(W_a, dtype=np.float32))
    U = np.ascontiguousarray(np.asarray(U_a, dtype=np.float32))
    V = np.asarray(V_a, dtype=np.float32).reshape(DE)

    # 32-column shifted-window weights: for chunk k, window
    # vz[:, k*63 + 31-i : k*63 + 63-i] has V_k in column i, zeros elsewhere.
    vz = np.zeros((128, KCH * 63), dtype=np.float32)
    for k in range(KCH):
        vz[:, k * 63 + 31] = V[k * 128:(k + 1) * 128]
    ident = np.eye(128, dtype=np.float32)

    in_maps = []
    for b in range(B):
        in_maps.append(
            {
                "encT": np.ascontiguousarray(enc_full[b].T),
                "decT": np.ascontiguousarray(dec_full[b].T),
                "enc": enc_full[b],
                "W": W,
                "U": U,
                "Vz": vz,
                "ident": ident,
            }
        )
    return in_maps


def run(inputs, trace=False):
    from concourse.bass_utils import run_bass_kernel_spmd

    nc = _get_program()
    in_maps = _prep_in_maps(**inputs)
    res = run_bass_kernel_spmd(nc, in_maps, list(range(B)), trace=trace)
    c = np.stack([res.results[b]["c_out"] for b in range(B)])
    e = np.stack([res.results[b]["e_out"] for b in range(B)])
    return (c, e), res


def predict_ns():
    """Cost-model timeline estimate for one core (no HW profiling needed)."""
    from concourse.timeline_sim import TimelineSim

    nc = _get_program()
    sim = TimelineSim(nc, trace=False)
    return sim.simulate()


def kernel(**inputs):
    out, _ = run(inputs, trace=False)
    return out
